# revision 3
# baseline (speedup 1.0000x reference)
"""Lorentz multi-head attention on 8 Trainium2 NeuronCores — v2.

Sharding: core c owns batch c//2 and heads (c%2)*4..+3 (1 batch x 4 heads),
so each core loads only its batch's x (5.2MB bf16) and runs packed
projection streams (two 64-wide heads stacked into one 128-wide stationary;
the second half is moved to partition base 0 with a cross-lane DVE copy).

Phase 1 per head: augmented-row Lorentz scores (row 64 = +-t gives
q.k - t_q t_k in one matmul), unnormalized exp attention (the Lorentz
midpoint renormalization absorbs the softmax denominator), and the
midpoint normalize directly in the d-major [65, N] layout: the colsum
weight vector [-1...-1,+1] yields r = t^2-||s||^2 in one matmul, and the
per-token rsqrt is broadcast across partitions via a stride-0 DRAM read
(no transposes).

Exchange: output tokens are striped so core c owns tokens [c*256,(c+1)*256)
of EVERY batch; one 8-way AllToAll per head-slot fires as soon as that
head's normalized midpoint is ready, overlapping later heads' compute.

Phase 2 stays d-major: the fused [640, 1024] operand is assembled purely
by DMA engines (head blocks straddle 128-partition chunks), the output
LorentzFC runs transposed (woT stationary f32r, fused moving), and y is
stored [513, 4*256] d-major; the host transposes.

sqrt = exp(0.5*ln(x)) keeps a single activation table set resident.
"""

import sys

sys.path.insert(0, "/opt/trn_rl_repo")

import numpy as np

import concourse.bass as bass
import concourse.mybir as mybir
import concourse.tile as tile
from concourse import bacc, bass_utils
from concourse.masks import make_identity

B, N, D = 4, 2048, 513
H, DHS = 8, 64
NCORES = 8
KCURV = 0.1
INVK = 10.0
SCALE = 1.0 / np.sqrt(DHS)
S_CONST = 2.8479428291320801  # exp(0.5*(digamma(256)-digamma(32)))
DPAD = 640
KC = 5
RPC = 1024  # tokens per core in phase 2 (4 batches x 256)
F32 = mybir.dt.float32
F32R = mybir.dt.float32r
BF16 = mybir.dt.bfloat16
Ln = mybir.ActivationFunctionType.Ln
Exp = mybir.ActivationFunctionType.Exp
MUL = mybir.AluOpType.mult

_CACHE = {}

# stream -> (kind, pos): S0=qkA S1=vAB S2=qkB S3=qkC S4=vCD S5=qkD
STREAMS = [("qk", 0), ("vv", 0), ("qk", 1), ("qk", 2), ("vv", 2), ("qk", 3)]


def _patch_act_tables(nc):
    # Keep Exp and Ln in the same table set so no reloads occur.
    from concourse.hw_specs import get_activation_tables

    try:
        tabs = get_activation_tables(nc.m.arch)
    except Exception:
        return
    if "natural_log_exp_and_others" not in tabs:
        return
    for name, fns in tabs.items():
        if name != "natural_log_exp_and_others":
            fns.discard(Exp)
            fns.discard(Ln)


def _r(ap):
    return ap.bitcast(F32R)


def _build():
    nc = bacc.Bacc(
        "TRN2", target_bir_lowering=False, debug=False, num_devices=NCORES
    )
    _patch_act_tables(nc)

    xT_ap = nc.dram_tensor("xT", [DPAD, N], BF16, kind="ExternalInput").ap()
    ws_ap = nc.dram_tensor("ws", [6, DPAD, 128], BF16, kind="ExternalInput").ap()
    woT_ap = nc.dram_tensor("woT", [DPAD, 512], F32, kind="ExternalInput").ap()
    sgn_ap = nc.dram_tensor("sgn", [2, 1], F32, kind="ExternalInput").ap()
    y_ap = nc.dram_tensor("y", [D, RPC], F32, kind="ExternalOutput").ap()

    with tile.TileContext(nc) as tc:
        with (
            tc.tile_pool(name="const", bufs=1) as constp,
            tc.tile_pool(name="w", bufs=1) as wp,
            tc.tile_pool(name="xT", bufs=1) as xtp,
            tc.tile_pool(name="qk", bufs=2) as qkp,
            tc.tile_pool(name="va", bufs=1) as vap,
            tc.tile_pool(name="pt", bufs=3) as ptp,
            tc.tile_pool(name="sm", bufs=2) as smp,
            tc.tile_pool(name="mid", bufs=1) as midp,
            tc.tile_pool(name="f2", bufs=1) as f2p,
            tc.tile_pool(name="ps", bufs=3, space="PSUM") as psp,
            tc.tile_pool(name="pj", bufs=1, space="PSUM") as pjp,
            tc.tile_pool(name="acc", bufs=1, space="PSUM") as accp,
            tc.tile_pool(name="dram", bufs=1, space="DRAM") as dramp,
        ):
            identb = constp.tile([64, 64], BF16)
            make_identity(nc, identb[:])
            ones8 = constp.tile([8, 1], BF16)
            nc.vector.memset(ones8[:], 1.0)
            ones128 = constp.tile([128, 1], BF16)
            nc.vector.memset(ones128[:], 1.0)
            onesrow = constp.tile([1, 256], F32)
            nc.vector.memset(onesrow[:], 1.0)
            zoner = constp.tile([128, 256], F32)
            nc.vector.memset(zoner[:], 0.0)
            bones = constp.tile([128, 2], BF16)
            nc.vector.memset(bones[:], 0.0)
            nc.vector.memset(bones[0:64, 0:1], 1.0)
            nc.vector.memset(bones[64:128, 1:2], 1.0)
            w65 = constp.tile([65, 1], BF16)
            nc.vector.memset(w65[:], -1.0)
            nc.vector.memset(w65[64:65, :], 1.0)
            sgn2 = constp.tile([2, 1], F32)
            nc.sync.dma_start(sgn2[:], sgn_ap)
            bias10 = constp.tile([128, 1], F32)
            nc.vector.memset(bias10[:], INVK)
            biasD = constp.tile([128, 1], F32)
            nc.vector.memset(biasD[:], INVK * (1.0 + H * S_CONST * S_CONST))

            ws = wp.tile([128, 6, KC, 128], BF16)
            ws_src = ws_ap.rearrange("s (k p) m -> s p k m", p=128)
            nc.sync.dma_start(ws[:, 0, :, :], ws_src[0])
            xt = []
            for ki in range(KC):
                t = xtp.tile([128, N], BF16, name=f"xt{ki}")
                nc.sync.dma_start(t[:], xT_ap[ki * 128 : (ki + 1) * 128, :])
                xt.append(t)
            for s in range(1, 6):
                nc.sync.dma_start(ws[:, s, :, :], ws_src[s])
            wo = wp.tile([128, KC, 512], F32R)
            nc.sync.dma_start(
                wo[:], woT_ap.rearrange("(k p) m -> p k m", p=128).bitcast(F32R)
            )

            send = [dramp.tile([8, 65, 256], F32, name=f"send{j}") for j in range(4)]
            recv = [dramp.tile([8, 65, 256], F32, name=f"recv{j}") for j in range(4)]
            rds = [dramp.tile([16, 128], F32, name=f"rd{i}") for i in range(2)]
            rvds = [dramp.tile([16, 128], F32, name=f"rvd{i}") for i in range(2)]
            dums = dramp.tile([8, 32], F32, name="dums")
            dumr = dramp.tile([8, 32], F32, name="dumr")
            nc.sync.dma_start(dums[:], zoner[0:8, 0:32])
            nc.gpsimd.collective_compute(
                "AllToAll",
                mybir.AluOpType.bypass,
                replica_groups=[list(range(NCORES))],
                ins=[dums.opt()],
                outs=[dumr.opt()],
            )

            qa, ka, vaT = {}, {}, {}
            va = [vap.tile([128, N // 128, 65], BF16, name=f"va{h}") for h in range(4)]

            def do_stream(s_idx):
                kind, pos = STREAMS[s_idx]
                if kind == "qk":
                    qa[pos] = qkp.tile([65, N], BF16, tag="qa", name=f"qa{pos}", bufs=3)
                    ka[pos] = qkp.tile([65, N], BF16, tag="ka", name=f"ka{pos}", bufs=3)
                    TL = smp.tile([2, N], F32, tag="TL", bufs=1)
                else:
                    vaT[pos] = qkp.tile([64, N], BF16, tag="vaT", name=f"vaT{pos}")
                    vaT[pos + 1] = qkp.tile(
                        [64, N], BF16, tag="vaT", name=f"vaT{pos + 1}"
                    )
                for nj in range(N // 512):
                    ps = pjp.tile([128, 512], F32, tag="pj")
                    for ki in range(KC):
                        nc.tensor.matmul(
                            ps[:],
                            ws[:, s_idx, ki, :],
                            xt[ki][:, nj * 512 : (nj + 1) * 512],
                            start=(ki == 0),
                            stop=(ki == KC - 1),
                        )
                    sl = slice(nj * 512, (nj + 1) * 512)
                    if kind == "qk":
                        # bf16 mirror of the packed projection output
                        pk = smp.tile([128, 512], BF16, tag="pk", bufs=3)
                        nc.vector.tensor_copy(pk[:], ps[:])
                        nc.vector.tensor_copy(qa[pos][0:64, sl], pk[0:64, :])
                        nc.vector.tensor_copy(ka[pos][0:64, sl], pk[64:128, :])
                        # t^2 sums: square + block colsum, ln from PSUM
                        sq = smp.tile([128, 512], BF16, tag="sq", bufs=3)
                        nc.vector.tensor_mul(sq[:], pk[:], pk[:])
                        psq = pjp.tile([2, 512], F32, tag="pj")
                        nc.tensor.matmul(psq[:], bones[:], sq[:], start=True, stop=True)
                        nc.scalar.activation(
                            TL[:, sl], psq[:], Ln, bias=bias10[0:2, :]
                        )
                    else:
                        nc.vector.tensor_copy(vaT[pos][:, sl], ps[0:64, :])
                        nc.vector.tensor_copy(vaT[pos + 1][:, sl], ps[64:128, :])
                if kind == "qk":
                    # t = exp(0.5*ln(.)), k row negated; rows -> qa/ka row 64
                    TE = smp.tile([2, N], F32, tag="TE", bufs=1)
                    nc.scalar.activation(TE[:], TL[:], Exp, scale=0.5)
                    TTbs = smp.tile([2, N], BF16, tag="TTbs", bufs=1)
                    nc.vector.tensor_scalar_mul(TTbs[:], TE[:], sgn2[:])
                    nc.sync.dma_start(qa[pos][64:65, :], TTbs[0:1, :])
                    nc.sync.dma_start(ka[pos][64:65, :], TTbs[1:2, :])

            def va_build(pos):
                # transpose vaT [64, N] into token-major va[:, :, 0:64]
                for half in range(2):
                    ptr = pjp.tile([128, 8, 64], BF16, tag="pj")
                    for j in range(8):
                        mi = half * 8 + j
                        nc.tensor.transpose(
                            ptr[:, j, :],
                            vaT[pos][:, mi * 128 : (mi + 1) * 128],
                            identb[:],
                        )
                    nc.vector.tensor_copy(
                        va[pos][:, half * 8 : half * 8 + 8, 0:64], ptr[:]
                    )
                # t_v token-major: accumulate squares along free dim
                vts = smp.tile([128, N // 128], F32, tag="vts", bufs=2)
                for mi in range(N // 128):
                    sqv = smp.tile([128, 64], BF16, tag="sqv", bufs=2)
                    nc.vector.scalar_tensor_tensor(
                        sqv[:],
                        va[pos][:, mi, 0:64],
                        1.0,
                        va[pos][:, mi, 0:64],
                        MUL,
                        MUL,
                        accum_out=vts[:, mi : mi + 1],
                    )
                lnv = smp.tile([128, N // 128], F32, tag="lnv", bufs=2)
                nc.scalar.activation(lnv[:], vts[:], Ln, bias=bias10[:])
                nc.scalar.activation(va[pos][:, :, 64:65], lnv[:], Exp, scale=0.5)

            def attention(pos):
                mts = [
                    accp.tile([65, 512], F32, tag=f"acc{nj}", name=f"mts{pos}_{nj}")
                    for nj in range(4)
                ]
                for mi in range(N // 128):
                    pt = ptp.tile([128, N], BF16, tag="pt")
                    for nj in range(4):
                        pss = psp.tile([128, 512], F32, tag="ps")
                        nc.tensor.matmul(
                            pss[:],
                            ka[pos][:, mi * 128 : (mi + 1) * 128],
                            qa[pos][:, nj * 512 : (nj + 1) * 512],
                            start=True,
                            stop=True,
                        )
                        nc.scalar.activation(
                            pt[:, nj * 512 : (nj + 1) * 512], pss[:], Exp, scale=SCALE
                        )
                    for nj in range(4):
                        nc.tensor.matmul(
                            mts[nj][:],
                            va[pos][:, mi, :],
                            pt[:, nj * 512 : (nj + 1) * 512],
                            start=(mi == 0),
                            stop=(mi == N // 128 - 1),
                        )
                return mts

            def midpoint(pos, mts):
                # mts rows: 0-63 spatial, 64 = time
                mT = midp.tile([65, N], F32, tag="mT")
                sqm = midp.tile([65, N], BF16, tag="sqm")
                rrow = smp.tile([1, N], F32, tag="rrow", bufs=1)
                for nj in range(4):
                    sl = slice(nj * 512, (nj + 1) * 512)
                    nc.vector.tensor_copy(mT[:, sl], mts[nj][:])
                    nc.vector.tensor_mul(sqm[:, sl], mT[:, sl], mT[:, sl])
                    # r = t^2 - ||s||^2 via weights (-1,...,-1,+1)
                    psc = pjp.tile([1, 512], F32, tag="pj")
                    nc.tensor.matmul(psc[:], w65[:], sqm[:, sl], start=True, stop=True)
                    nc.vector.tensor_copy(rrow[:, sl], psc[:])
                # rinv = exp(-0.5*ln(K*r)) computed in [16, 128] layout
                rd, rvd = rds[pos % 2], rvds[pos % 2]
                nc.sync.dma_start(rd[:].rearrange("p f -> (p f)"), rrow[:])
                r16 = smp.tile([16, 128], F32, tag="r16", bufs=2)
                nc.sync.dma_start(r16[:], rd[:])
                ln16 = smp.tile([16, 128], F32, tag="ln16", bufs=2)
                nc.scalar.activation(ln16[:], r16[:], Ln, scale=KCURV)
                rv16 = smp.tile([16, 128], F32, tag="rv16", bufs=2)
                nc.scalar.activation(rv16[:], ln16[:], Exp, scale=-0.5)
                nc.sync.dma_start(rvd[:], rv16[:])
                rinvB = midp.tile([65, N], F32, tag="rinvB")
                rsrc = rvd[:]
                nc.sync.dma_start(
                    rinvB[:], bass.AP(rsrc.tensor, rsrc.offset, [[0, 65], [1, N]])
                )
                moT = midp.tile([65, N], F32, tag="moT")
                nc.vector.tensor_mul(moT[:], mT[:], rinvB[:])
                nc.sync.dma_start(
                    send[pos][:].rearrange("c r t -> r c t"),
                    moT[:].rearrange("r (c t) -> r c t", c=8),
                )
                nc.gpsimd.collective_compute(
                    "AllToAll",
                    mybir.AluOpType.bypass,
                    replica_groups=[list(range(NCORES))],
                    ins=[send[pos].opt()],
                    outs=[recv[pos].opt()],
                )

            # ---------------- phase 1 schedule ----------------
            do_stream(0)
            do_stream(1)
            va_build(0)
            va_build(1)
            mts0 = attention(0)
            midpoint(0, mts0)
            do_stream(2)
            do_stream(3)
            do_stream(4)
            do_stream(5)
            mts1 = attention(1)
            midpoint(1, mts1)
            va_build(2)
            va_build(3)
            mts2 = attention(2)
            midpoint(2, mts2)
            mts3 = attention(3)
            midpoint(3, mts3)

            # ---------------- phase 2 ----------------
            # All 4 batches at once: fuT [128, KC, 4, 256]; this core owns
            # tokens [my_idx*256, +256) of every batch. For head h the 4
            # batches live at recv[h%4] chunks (2*tb + h//4).
            fuT = f2p.tile([128, KC, 4, 256], F32R, bufs=1, name="fuT")
            zsrc = zoner[:].bitcast(F32R)
            nc.sync.dma_start(
                fuT[:, 4, :, :],
                bass.AP(zsrc.tensor, zsrc.offset, [list(zsrc.ap[0]), [0, 4], [1, 256]]),
            )
            osrc = onesrow[:].bitcast(F32R)
            nc.sync.dma_start(
                fuT[1:2, 4, :, :],
                bass.AP(osrc.tensor, osrc.offset, [list(osrc.ap[0]), [0, 4], [1, 256]]),
            )
            trows = f2p.tile([8, 4, 256], F32, bufs=1, name="trows")
            for h in range(8):
                j, s0 = h % 4, h // 4
                d0 = 1 + 64 * h
                src_sp = recv[j][s0::2, 0:64, :].rearrange("s r t -> r s t")
                src_sp = src_sp.bitcast(F32R)
                c0, p0 = divmod(d0, 128)
                n0 = min(64, 128 - p0)
                nc.sync.dma_start(fuT[p0 : p0 + n0, c0, :, :], src_sp[0:n0])
                if n0 < 64:
                    nc.sync.dma_start(fuT[0 : 64 - n0, c0 + 1, :, :], src_sp[n0:64])
                nc.sync.dma_start(
                    trows[h : h + 1, :, :],
                    recv[j][s0::2, 64:65, :].rearrange("s r t -> r s t"),
                )
            # t' = sqrt(biasD + S^2 * sum_h t_h^2)
            sqt = smp.tile([8, 4, 256], BF16, tag="sqt")
            nc.vector.tensor_mul(sqt[:], trows[:], trows[:])
            sqtf = sqt[:].rearrange("p b t -> p (b t)")
            fuflat = fuT[:].rearrange("p k b t -> p k (b t)")
            for half in range(2):
                fsl = slice(half * 512, (half + 1) * 512)
                pst = pjp.tile([1, 512], F32, tag="pj")
                nc.tensor.matmul(pst[:], ones8[:], sqtf[:, fsl], start=True, stop=True)
                lnt = smp.tile([1, 512], F32, tag="lnt", bufs=2)
                nc.scalar.activation(
                    lnt[:], pst[:], Ln, scale=S_CONST * S_CONST, bias=biasD[0:1, :]
                )
                nc.scalar.activation(fuflat[0:1, 0, fsl], lnt[:], Exp, scale=0.5)

            pstq = [
                accp.tile([1, 512], F32, tag=f"acc{i}", name=f"tq{i}") for i in range(2)
            ]
            for oc in range(4):
                for half in range(2):
                    fsl = slice(half * 512, (half + 1) * 512)
                    pso = psp.tile([128, 512], F32, tag="ps")
                    for ki in range(KC):
                        nc.tensor.matmul(
                            pso[:],
                            wo[:, ki, oc * 128 : (oc + 1) * 128],
                            fuflat[:, ki, fsl],
                            start=(ki == 0),
                            stop=(ki == KC - 1),
                        )
                    outt = f2p.tile([128, 512], F32, tag="outt", bufs=3)
                    nc.vector.tensor_copy(outt[:], pso[:])
                    sqo = smp.tile([128, 512], BF16, tag="sqo", bufs=3)
                    nc.vector.tensor_mul(sqo[:], outt[:], outt[:])
                    nc.tensor.matmul(
                        pstq[half][:],
                        ones128[:],
                        sqo[:],
                        start=(oc == 0),
                        stop=(oc == 3),
                    )
                    nc.sync.dma_start(
                        y_ap[1 + oc * 128 : 1 + (oc + 1) * 128, fsl], outt[:]
                    )
            for half in range(2):
                fsl = slice(half * 512, (half + 1) * 512)
                lno = smp.tile([1, 512], F32, tag="lno", bufs=2)
                nc.scalar.activation(lno[:], pstq[half][:], Ln, bias=bias10[0:1, :])
                tor = smp.tile([1, 512], F32, tag="tor", bufs=2)
                nc.scalar.activation(tor[:], lno[:], Exp, scale=0.5)
                nc.sync.dma_start(y_ap[0:1, fsl], tor[:])

    nc.compile()
    return nc


def _prep_inputs(x, Wq, bq, Wk, bk, Wv, bv, Wo, bo):
    import ml_dtypes

    bf = ml_dtypes.bfloat16
    woT = np.zeros((DPAD, 512), dtype=np.float32)
    woT[0] = Wo[:, 0]
    woT[1:D] = Wo[:, 1:].T * S_CONST
    woT[D] = bo
    sgn = np.array([[1.0], [-1.0]], dtype=np.float32)

    in_maps = []
    for c in range(NCORES):
        bc = c // 2
        hs = [(c % 2) * 4 + i for i in range(4)]
        xT = np.zeros((DPAD, N), dtype=np.float32)
        xT[:D] = x[bc].T
        xT[D] = 1.0

        def wpack(Wa, ba, Wb, bb):
            w = np.zeros((DPAD, 128), dtype=np.float32)
            w[:D, 0:64] = Wa.T
            w[D, 0:64] = ba
            w[:D, 64:128] = Wb.T
            w[D, 64:128] = bb
            return w

        ws = np.stack([
            wpack(Wq[hs[0]], bq[hs[0]], Wk[hs[0]], bk[hs[0]]),
            wpack(Wv[hs[0]], bv[hs[0]], Wv[hs[1]], bv[hs[1]]),
            wpack(Wq[hs[1]], bq[hs[1]], Wk[hs[1]], bk[hs[1]]),
            wpack(Wq[hs[2]], bq[hs[2]], Wk[hs[2]], bk[hs[2]]),
            wpack(Wv[hs[2]], bv[hs[2]], Wv[hs[3]], bv[hs[3]]),
            wpack(Wq[hs[3]], bq[hs[3]], Wk[hs[3]], bk[hs[3]]),
        ])
        in_maps.append({
            "xT": xT.astype(bf),
            "ws": ws.astype(bf),
            "woT": woT,
            "sgn": sgn,
        })
    return in_maps


def _run(inputs, trace=False, **kw):
    if "nc" not in _CACHE:
        _CACHE["nc"] = _build()
    nc = _CACHE["nc"]
    in_maps = _prep_inputs(**{k: np.asarray(v) for k, v in inputs.items()})
    res = bass_utils.run_bass_kernel_spmd(
        nc, in_maps, core_ids=list(range(NCORES)), trace=trace, **kw
    )
    y = np.zeros((B, N, D), dtype=np.float32)
    for c in range(NCORES):
        yc = res.results[c]["y"]  # [D, 4*256]: batch-major columns
        for b in range(B):
            y[b, c * 256 : (c + 1) * 256] = yc[:, b * 256 : (b + 1) * 256].T
    return y, res


def kernel(**inputs):
    y, _ = _run(inputs)
    return y


# revision 4
# speedup vs baseline: 1.2330x; 1.2330x over previous
"""Lorentz multi-head attention on 8 Trainium2 NeuronCores — v2.

Sharding: core c owns batch c//2 and heads (c%2)*4..+3 (1 batch x 4 heads),
so each core loads only its batch's x (5.2MB bf16) and runs packed
projection streams (two 64-wide heads stacked into one 128-wide stationary;
the second half is moved to partition base 0 with a cross-lane DVE copy).

Phase 1 per head: augmented-row Lorentz scores (row 64 = +-t gives
q.k - t_q t_k in one matmul), unnormalized exp attention (the Lorentz
midpoint renormalization absorbs the softmax denominator), and the
midpoint normalize directly in the d-major [65, N] layout: the colsum
weight vector [-1...-1,+1] yields r = t^2-||s||^2 in one matmul, and the
per-token rsqrt is broadcast across partitions via a stride-0 DRAM read
(no transposes).

Exchange: output tokens are striped so core c owns tokens [c*256,(c+1)*256)
of EVERY batch; one 8-way AllToAll per head-slot fires as soon as that
head's normalized midpoint is ready, overlapping later heads' compute.

Phase 2 stays d-major: the fused [640, 1024] operand is assembled purely
by DMA engines (head blocks straddle 128-partition chunks), the output
LorentzFC runs transposed (woT stationary f32r, fused moving), and y is
stored [513, 4*256] d-major; the host transposes.

sqrt = exp(0.5*ln(x)) keeps a single activation table set resident.
"""

import sys

sys.path.insert(0, "/opt/trn_rl_repo")

import numpy as np

import concourse.bass as bass
import concourse.mybir as mybir
import concourse.tile as tile
from concourse import bacc, bass_utils
from concourse.masks import make_identity

B, N, D = 4, 2048, 513
H, DHS = 8, 64
NCORES = 8
KCURV = 0.1
INVK = 10.0
SCALE = 1.0 / np.sqrt(DHS)
S_CONST = 2.8479428291320801  # exp(0.5*(digamma(256)-digamma(32)))
DPAD = 640
KC = 5
RPC = 1024  # tokens per core in phase 2 (4 batches x 256)
F32 = mybir.dt.float32
F32R = mybir.dt.float32r
BF16 = mybir.dt.bfloat16
Ln = mybir.ActivationFunctionType.Ln
Exp = mybir.ActivationFunctionType.Exp
MUL = mybir.AluOpType.mult

_CACHE = {}

# stream -> (kind, pos): S0=qkA S1=vAB S2=qkB S3=qkC S4=vCD S5=qkD
STREAMS = [("qk", 0), ("vv", 0), ("qk", 1), ("qk", 2), ("vv", 2), ("qk", 3)]


def _patch_act_tables(nc):
    # Keep Exp and Ln in the same table set so no reloads occur.
    from concourse.hw_specs import get_activation_tables

    try:
        tabs = get_activation_tables(nc.m.arch)
    except Exception:
        return
    if "natural_log_exp_and_others" not in tabs:
        return
    for name, fns in tabs.items():
        if name != "natural_log_exp_and_others":
            fns.discard(Exp)
            fns.discard(Ln)


def _r(ap):
    return ap.bitcast(F32R)


def _build():
    nc = bacc.Bacc(
        "TRN2", target_bir_lowering=False, debug=False, num_devices=NCORES
    )
    _patch_act_tables(nc)

    xT_ap = nc.dram_tensor("xT", [DPAD, N], BF16, kind="ExternalInput").ap()
    ws_ap = nc.dram_tensor("ws", [6, DPAD, 128], BF16, kind="ExternalInput").ap()
    woT_ap = nc.dram_tensor("woT", [DPAD, 512], F32, kind="ExternalInput").ap()
    sgn_ap = nc.dram_tensor("sgn", [2, 1], F32, kind="ExternalInput").ap()
    y_ap = nc.dram_tensor("y", [D, RPC], F32, kind="ExternalOutput").ap()

    with tile.TileContext(nc) as tc:
        with (
            tc.tile_pool(name="const", bufs=1) as constp,
            tc.tile_pool(name="w", bufs=1) as wp,
            tc.tile_pool(name="xT", bufs=1) as xtp,
            tc.tile_pool(name="qk", bufs=2) as qkp,
            tc.tile_pool(name="va", bufs=1) as vap,
            tc.tile_pool(name="pt", bufs=2) as ptp,
            tc.tile_pool(name="sm", bufs=2) as smp,
            tc.tile_pool(name="mid", bufs=1) as midp,
            tc.tile_pool(name="f2", bufs=1) as f2p,
            tc.tile_pool(name="ps", bufs=3, space="PSUM") as psp,
            tc.tile_pool(name="pj", bufs=1, space="PSUM") as pjp,
            tc.tile_pool(name="acc", bufs=1, space="PSUM") as accp,
            tc.tile_pool(name="dram", bufs=1, space="DRAM") as dramp,
        ):
            identb = constp.tile([64, 64], BF16)
            make_identity(nc, identb[:])
            ones8 = constp.tile([8, 1], BF16)
            nc.vector.memset(ones8[:], 1.0)
            ones128 = constp.tile([128, 1], BF16)
            nc.vector.memset(ones128[:], 1.0)
            onesrow = constp.tile([1, 256], F32)
            nc.vector.memset(onesrow[:], 1.0)
            zoner = constp.tile([128, 256], F32)
            nc.vector.memset(zoner[:], 0.0)
            bones = constp.tile([128, 2], BF16)
            nc.vector.memset(bones[:], 0.0)
            nc.vector.memset(bones[0:64, 0:1], 1.0)
            nc.vector.memset(bones[64:128, 1:2], 1.0)
            w65 = constp.tile([65, 1], BF16)
            nc.vector.memset(w65[:], -1.0)
            nc.vector.memset(w65[64:65, :], 1.0)
            sgn2 = constp.tile([2, 1], F32)
            nc.sync.dma_start(sgn2[:], sgn_ap)
            bias10 = constp.tile([128, 1], F32)
            nc.vector.memset(bias10[:], INVK)
            biasD = constp.tile([128, 1], F32)
            nc.vector.memset(biasD[:], INVK * (1.0 + H * S_CONST * S_CONST))

            ws = wp.tile([128, 6, KC, 128], BF16)
            ws_src = ws_ap.rearrange("s (k p) m -> s p k m", p=128)
            nc.sync.dma_start(ws[:, 0, :, :], ws_src[0])
            xt = []
            for ki in range(KC):
                t = xtp.tile([128, N], BF16, name=f"xt{ki}")
                nc.sync.dma_start(t[:], xT_ap[ki * 128 : (ki + 1) * 128, :])
                xt.append(t)
            for s in range(1, 6):
                nc.sync.dma_start(ws[:, s, :, :], ws_src[s])
            wo = wp.tile([128, KC, 512], F32R)
            nc.sync.dma_start(
                wo[:], woT_ap.rearrange("(k p) m -> p k m", p=128).bitcast(F32R)
            )

            send = [dramp.tile([8, 65, 256], F32, name=f"send{j}") for j in range(4)]
            recv = [dramp.tile([8, 65, 256], F32, name=f"recv{j}") for j in range(4)]
            rds = [dramp.tile([16, 128], F32, name=f"rd{i}") for i in range(2)]
            rvds = [dramp.tile([16, 128], F32, name=f"rvd{i}") for i in range(2)]
            dums = dramp.tile([8, 32], F32, name="dums")
            dumr = dramp.tile([8, 32], F32, name="dumr")
            nc.sync.dma_start(dums[:], zoner[0:8, 0:32])
            nc.gpsimd.collective_compute(
                "AllToAll",
                mybir.AluOpType.bypass,
                replica_groups=[list(range(NCORES))],
                ins=[dums.opt()],
                outs=[dumr.opt()],
            )

            qa, ka, vaT = {}, {}, {}
            va = [vap.tile([128, N // 128, 65], BF16, name=f"va{h}") for h in range(4)]

            def do_stream(s_idx):
                kind, pos = STREAMS[s_idx]
                if kind == "qk":
                    qa[pos] = qkp.tile([65, N], BF16, tag="qa", name=f"qa{pos}", bufs=3)
                    ka[pos] = qkp.tile([65, N], BF16, tag="ka", name=f"ka{pos}", bufs=3)
                    TL = smp.tile([2, N], F32, tag="TL", bufs=1)
                else:
                    vaT[pos] = qkp.tile([64, N], BF16, tag="vaT", name=f"vaT{pos}")
                    vaT[pos + 1] = qkp.tile(
                        [64, N], BF16, tag="vaT", name=f"vaT{pos + 1}"
                    )
                for nj in range(N // 512):
                    ps = pjp.tile([128, 512], F32, tag="pj")
                    for ki in range(KC):
                        nc.tensor.matmul(
                            ps[:],
                            ws[:, s_idx, ki, :],
                            xt[ki][:, nj * 512 : (nj + 1) * 512],
                            start=(ki == 0),
                            stop=(ki == KC - 1),
                        )
                    sl = slice(nj * 512, (nj + 1) * 512)
                    if kind == "qk":
                        # bf16 mirror of the packed projection output
                        pk = smp.tile([128, 512], BF16, tag="pk", bufs=3)
                        nc.vector.tensor_copy(pk[:], ps[:])
                        nc.vector.tensor_copy(qa[pos][0:64, sl], pk[0:64, :])
                        nc.vector.tensor_copy(ka[pos][0:64, sl], pk[64:128, :])
                        # t^2 sums: square + block colsum, ln from PSUM
                        sq = smp.tile([128, 512], BF16, tag="sq", bufs=3)
                        nc.vector.tensor_mul(sq[:], pk[:], pk[:])
                        psq = pjp.tile([2, 512], F32, tag="pj")
                        nc.tensor.matmul(psq[:], bones[:], sq[:], start=True, stop=True)
                        nc.scalar.activation(
                            TL[:, sl], psq[:], Ln, bias=bias10[0:2, :]
                        )
                    else:
                        nc.vector.tensor_copy(vaT[pos][:, sl], ps[0:64, :])
                        nc.vector.tensor_copy(vaT[pos + 1][:, sl], ps[64:128, :])
                if kind == "qk":
                    # t = exp(0.5*ln(.)), k row negated; rows -> qa/ka row 64
                    TE = smp.tile([2, N], F32, tag="TE", bufs=1)
                    nc.scalar.activation(TE[:], TL[:], Exp, scale=0.5)
                    TTbs = smp.tile([2, N], BF16, tag="TTbs", bufs=1)
                    nc.vector.tensor_scalar_mul(TTbs[:], TE[:], sgn2[:])
                    nc.sync.dma_start(qa[pos][64:65, :], TTbs[0:1, :])
                    nc.sync.dma_start(ka[pos][64:65, :], TTbs[1:2, :])

            def va_build(pos):
                # transpose vaT [64, N] into token-major va[:, :, 0:64]
                for half in range(2):
                    ptr = pjp.tile([128, 8, 64], BF16, tag="pj")
                    for j in range(8):
                        mi = half * 8 + j
                        nc.tensor.transpose(
                            ptr[:, j, :],
                            vaT[pos][:, mi * 128 : (mi + 1) * 128],
                            identb[:],
                        )
                    nc.vector.tensor_copy(
                        va[pos][:, half * 8 : half * 8 + 8, 0:64], ptr[:]
                    )
                # t_v token-major: accumulate squares along free dim
                vts = smp.tile([128, N // 128], F32, tag="vts", bufs=2)
                for mi in range(N // 128):
                    sqv = smp.tile([128, 64], BF16, tag="sqv", bufs=2)
                    nc.vector.scalar_tensor_tensor(
                        sqv[:],
                        va[pos][:, mi, 0:64],
                        1.0,
                        va[pos][:, mi, 0:64],
                        MUL,
                        MUL,
                        accum_out=vts[:, mi : mi + 1],
                    )
                lnv = smp.tile([128, N // 128], F32, tag="lnv", bufs=2)
                nc.scalar.activation(lnv[:], vts[:], Ln, bias=bias10[:])
                nc.scalar.activation(va[pos][:, :, 64:65], lnv[:], Exp, scale=0.5)

            def attention(pos):
                mts = [
                    accp.tile([65, 512], F32, tag=f"acc{nj}", name=f"mts{pos}_{nj}")
                    for nj in range(4)
                ]
                for mi in range(N // 128):
                    pt = ptp.tile([128, N], BF16, tag="pt")
                    for nj in range(4):
                        pss = psp.tile([128, 512], F32, tag="ps")
                        nc.tensor.matmul(
                            pss[:],
                            ka[pos][:, mi * 128 : (mi + 1) * 128],
                            qa[pos][:, nj * 512 : (nj + 1) * 512],
                            start=True,
                            stop=True,
                        )
                        nc.scalar.activation(
                            pt[:, nj * 512 : (nj + 1) * 512], pss[:], Exp, scale=SCALE
                        )
                    for nj in range(4):
                        nc.tensor.matmul(
                            mts[nj][:],
                            va[pos][:, mi, :],
                            pt[:, nj * 512 : (nj + 1) * 512],
                            start=(mi == 0),
                            stop=(mi == N // 128 - 1),
                        )
                return mts

            def midpoint(pos, mts):
                # mts rows: 0-63 spatial, 64 = time
                mT = midp.tile([65, N], F32, tag="mT")
                sqm = midp.tile([65, N], BF16, tag="sqm")
                rrow = smp.tile([1, N], F32, tag="rrow", bufs=1)
                for nj in range(4):
                    sl = slice(nj * 512, (nj + 1) * 512)
                    nc.vector.tensor_copy(mT[:, sl], mts[nj][:])
                    nc.vector.tensor_mul(sqm[:, sl], mT[:, sl], mT[:, sl])
                    # r = t^2 - ||s||^2 via weights (-1,...,-1,+1)
                    psc = pjp.tile([1, 512], F32, tag="pj")
                    nc.tensor.matmul(psc[:], w65[:], sqm[:, sl], start=True, stop=True)
                    nc.vector.tensor_copy(rrow[:, sl], psc[:])
                # rinv = exp(-0.5*ln(K*r)) computed in [16, 128] layout
                rd, rvd = rds[pos % 2], rvds[pos % 2]
                nc.sync.dma_start(rd[:].rearrange("p f -> (p f)"), rrow[:])
                r16 = smp.tile([16, 128], F32, tag="r16", bufs=2)
                nc.sync.dma_start(r16[:], rd[:])
                ln16 = smp.tile([16, 128], F32, tag="ln16", bufs=2)
                nc.scalar.activation(ln16[:], r16[:], Ln, scale=KCURV)
                rv16 = smp.tile([16, 128], F32, tag="rv16", bufs=2)
                nc.scalar.activation(rv16[:], ln16[:], Exp, scale=-0.5)
                nc.sync.dma_start(rvd[:], rv16[:])
                rinvB = midp.tile([65, N], F32, tag="rinvB")
                rsrc = rvd[:]
                nc.sync.dma_start(
                    rinvB[:], bass.AP(rsrc.tensor, rsrc.offset, [[0, 65], [1, N]])
                )
                moT = midp.tile([65, N], F32, tag="moT")
                nc.vector.tensor_mul(moT[:], mT[:], rinvB[:])
                nc.sync.dma_start(
                    send[pos][:].rearrange("c r t -> r c t"),
                    moT[:].rearrange("r (c t) -> r c t", c=8),
                )
                nc.gpsimd.collective_compute(
                    "AllToAll",
                    mybir.AluOpType.bypass,
                    replica_groups=[list(range(NCORES))],
                    ins=[send[pos].opt()],
                    outs=[recv[pos].opt()],
                )

            # ---------------- phase 1 schedule ----------------
            do_stream(0)
            do_stream(1)
            va_build(0)
            va_build(1)
            mts0 = attention(0)
            midpoint(0, mts0)
            do_stream(2)
            do_stream(3)
            do_stream(4)
            do_stream(5)
            mts1 = attention(1)
            midpoint(1, mts1)
            va_build(2)
            va_build(3)
            mts2 = attention(2)
            midpoint(2, mts2)
            mts3 = attention(3)
            midpoint(3, mts3)

            # ---------------- phase 2 ----------------
            # All 4 batches at once: fuT [128, KC, 4, 256]; this core owns
            # tokens [my_idx*256, +256) of every batch. For head h the 4
            # batches live at recv[h%4] chunks (2*tb + h//4).
            fuT = f2p.tile([128, KC, 4, 256], F32R, bufs=1, name="fuT")
            zsrc = zoner[:].bitcast(F32R)
            nc.sync.dma_start(
                fuT[:, 4, :, :],
                bass.AP(zsrc.tensor, zsrc.offset, [list(zsrc.ap[0]), [0, 4], [1, 256]]),
            )
            osrc = onesrow[:].bitcast(F32R)
            nc.sync.dma_start(
                fuT[1:2, 4, :, :],
                bass.AP(osrc.tensor, osrc.offset, [list(osrc.ap[0]), [0, 4], [1, 256]]),
            )
            trows = f2p.tile([8, 4, 256], F32, bufs=1, name="trows")
            for h in range(8):
                j, s0 = h % 4, h // 4
                d0 = 1 + 64 * h
                src_sp = recv[j][s0::2, 0:64, :].rearrange("s r t -> r s t")
                src_sp = src_sp.bitcast(F32R)
                c0, p0 = divmod(d0, 128)
                n0 = min(64, 128 - p0)
                nc.sync.dma_start(fuT[p0 : p0 + n0, c0, :, :], src_sp[0:n0])
                if n0 < 64:
                    nc.sync.dma_start(fuT[0 : 64 - n0, c0 + 1, :, :], src_sp[n0:64])
                nc.sync.dma_start(
                    trows[h : h + 1, :, :],
                    recv[j][s0::2, 64:65, :].rearrange("s r t -> r s t"),
                )
            # t' = sqrt(biasD + S^2 * sum_h t_h^2)
            sqt = smp.tile([8, 4, 256], BF16, tag="sqt")
            nc.vector.tensor_mul(sqt[:], trows[:], trows[:])
            sqtf = sqt[:].rearrange("p b t -> p (b t)")
            fuflat = fuT[:].rearrange("p k b t -> p k (b t)")
            for half in range(2):
                fsl = slice(half * 512, (half + 1) * 512)
                pst = pjp.tile([1, 512], F32, tag="pj")
                nc.tensor.matmul(pst[:], ones8[:], sqtf[:, fsl], start=True, stop=True)
                lnt = smp.tile([1, 512], F32, tag="lnt", bufs=2)
                nc.scalar.activation(
                    lnt[:], pst[:], Ln, scale=S_CONST * S_CONST, bias=biasD[0:1, :]
                )
                nc.scalar.activation(fuflat[0:1, 0, fsl], lnt[:], Exp, scale=0.5)

            pstq = [
                accp.tile([1, 512], F32, tag=f"acc{i}", name=f"tq{i}") for i in range(2)
            ]
            for oc in range(4):
                for half in range(2):
                    fsl = slice(half * 512, (half + 1) * 512)
                    pso = psp.tile([128, 512], F32, tag="ps")
                    for ki in range(KC):
                        nc.tensor.matmul(
                            pso[:],
                            wo[:, ki, oc * 128 : (oc + 1) * 128],
                            fuflat[:, ki, fsl],
                            start=(ki == 0),
                            stop=(ki == KC - 1),
                        )
                    outt = f2p.tile([128, 512], F32, tag="outt", bufs=3)
                    nc.vector.tensor_copy(outt[:], pso[:])
                    sqo = smp.tile([128, 512], BF16, tag="sqo", bufs=3)
                    nc.vector.tensor_mul(sqo[:], outt[:], outt[:])
                    nc.tensor.matmul(
                        pstq[half][:],
                        ones128[:],
                        sqo[:],
                        start=(oc == 0),
                        stop=(oc == 3),
                    )
                    nc.sync.dma_start(
                        y_ap[1 + oc * 128 : 1 + (oc + 1) * 128, fsl], outt[:]
                    )
            for half in range(2):
                fsl = slice(half * 512, (half + 1) * 512)
                lno = smp.tile([1, 512], F32, tag="lno", bufs=2)
                nc.scalar.activation(lno[:], pstq[half][:], Ln, bias=bias10[0:1, :])
                tor = smp.tile([1, 512], F32, tag="tor", bufs=2)
                nc.scalar.activation(tor[:], lno[:], Exp, scale=0.5)
                nc.sync.dma_start(y_ap[0:1, fsl], tor[:])

    nc.compile()
    return nc


def _prep_inputs(x, Wq, bq, Wk, bk, Wv, bv, Wo, bo):
    import ml_dtypes

    bf = ml_dtypes.bfloat16
    woT = np.zeros((DPAD, 512), dtype=np.float32)
    woT[0] = Wo[:, 0]
    woT[1:D] = Wo[:, 1:].T * S_CONST
    woT[D] = bo
    sgn = np.array([[1.0], [-1.0]], dtype=np.float32)

    in_maps = []
    for c in range(NCORES):
        bc = c // 2
        hs = [(c % 2) * 4 + i for i in range(4)]
        xT = np.zeros((DPAD, N), dtype=np.float32)
        xT[:D] = x[bc].T
        xT[D] = 1.0

        def wpack(Wa, ba, Wb, bb):
            w = np.zeros((DPAD, 128), dtype=np.float32)
            w[:D, 0:64] = Wa.T
            w[D, 0:64] = ba
            w[:D, 64:128] = Wb.T
            w[D, 64:128] = bb
            return w

        ws = np.stack([
            wpack(Wq[hs[0]], bq[hs[0]], Wk[hs[0]], bk[hs[0]]),
            wpack(Wv[hs[0]], bv[hs[0]], Wv[hs[1]], bv[hs[1]]),
            wpack(Wq[hs[1]], bq[hs[1]], Wk[hs[1]], bk[hs[1]]),
            wpack(Wq[hs[2]], bq[hs[2]], Wk[hs[2]], bk[hs[2]]),
            wpack(Wv[hs[2]], bv[hs[2]], Wv[hs[3]], bv[hs[3]]),
            wpack(Wq[hs[3]], bq[hs[3]], Wk[hs[3]], bk[hs[3]]),
        ])
        in_maps.append({
            "xT": xT.astype(bf),
            "ws": ws.astype(bf),
            "woT": woT,
            "sgn": sgn,
        })
    return in_maps


def _run(inputs, trace=False, **kw):
    if "nc" not in _CACHE:
        _CACHE["nc"] = _build()
    nc = _CACHE["nc"]
    in_maps = _prep_inputs(**{k: np.asarray(v) for k, v in inputs.items()})
    res = bass_utils.run_bass_kernel_spmd(
        nc, in_maps, core_ids=list(range(NCORES)), trace=trace, **kw
    )
    y = np.zeros((B, N, D), dtype=np.float32)
    for c in range(NCORES):
        yc = res.results[c]["y"]  # [D, 4*256]: batch-major columns
        for b in range(B):
            y[b, c * 256 : (c + 1) * 256] = yc[:, b * 256 : (b + 1) * 256].T
    return y, res


def kernel(**inputs):
    y, _ = _run(inputs)
    return y
